# revision 1
# baseline (speedup 1.0000x reference)
"""CenterNet decoder (top-100 detection) as a TRN2 Bass kernel, 8-core SPMD.

Contract: kernel(x, wh, reg) takes the FULL inputs
  x   [16, 80, 256, 256] f32   class heatmap scores
  wh  [16, 2, 256, 256]  f32   box sizes
  reg [16, 2, 256, 256]  f32   center offsets
and returns (topk_classes [16,100] f32, scores [16,100] f32,
results [16,100,4] f32) exactly like the reference CenterNet decoder
(top-k over flattened class*spatial scores, stable ties by index asc).

Sharding: pure data parallel, batch dim split 2 samples per core.

Per-core device pipeline (per sample; v = flat scores [5242880] viewed as
640 rows x 8192):
  A: stream 5 tiles [128, 8192]; per-row top-8 (DVE max8) -> cand [128, 40]
  B: m = per-partition top-8 of cand; 13 multiset-exact extraction rounds
     (max8 -> PE flatten -> max8 -> match_replace) -> gv = sorted top-104
     values. Only tau = gv[103] is consumed downstream; a duplicated value
     straddling a round boundary can drop a copy, which only lowers tau
     (more candidates marked; still correct).
  C: mark cand >= tau; per-partition counts; prefix sums via PE triangular
     matmuls -> per-compaction-slot source coordinates.
  D: stage (val, rowid) to DRAM; indirect-gather marked candidates into
     compact per-slot form (C <= 128 slots; huge statistical margin).
  E: indirect-gather each slot's source row; max_index with the value
     replicated 8x yields successive occurrence offsets (duplicate-safe).
  F: copy-ordinal (same value+row duplicates) selects the occurrence;
     flat index = rowid*8192 + offset.
  G: exact rank = #{v' > v} + #{v'==v and idx' < idx} via [128,128]
     compare masks + row-reduce (ties by index asc = jax.lax.top_k order).
  H: decode class/x/y/box with indirect-gathered reg/wh at the winners.
  I: indirect-scatter output rows at offset=rank (bounds drop rank >= 104).
"""
import os
import numpy as np

import concourse.bass as bass
import concourse.bacc as bacc
import concourse.mybir as mybir
from concourse.tile import TileContext
from concourse.bass_utils import run_bass_kernel_spmd

F32 = mybir.dt.float32
U32 = mybir.dt.uint32
OP = mybir.AluOpType

N_CORES = 8
B_FULL = 16
B_LOC = 2          # samples per core
ROWS = 640         # rows of 8192 per sample
TILE_F = 8192
N_TILES = 5
K_OUT = 104        # extract top-104 (>=100 needed; slack for threshold ties)
TOPK = 100
NEG = -1e30

LAST_EXEC_NS = None


def host_constants():
    i = np.arange(128)
    return {
        "c_identity": np.eye(128, dtype=np.float32),
        "c_ones8": np.ones((1, 8), dtype=np.float32),
        "c_sel8": np.eye(8, dtype=np.float32),
        "c_triu": (i[:, None] <= i[None, :]).astype(np.float32),
        "c_shiftup": (i[:, None] == i[None, :] + 1).astype(np.float32),
        "c_slt": (i[None, :] < i[:, None]).astype(np.float32),
        "c_irow": np.tile(i[None, :], (128, 1)).astype(np.float32),
        "c_iota08": np.tile(np.arange(8, dtype=np.float32)[None, :], (128, 1)),
        "c_iotap_f": i[:, None].astype(np.float32),
        "c_iotap_u": i[:, None].astype(np.uint32),
        "c_ones_col": np.ones((128, 1), dtype=np.float32),
    }


def build_kernel(n_cores=N_CORES):
    nc = bacc.Bacc("TRN2", target_bir_lowering=False, debug=False,
                   num_devices=n_cores)

    xs = nc.declare_dram_parameter("xs", [B_LOC * ROWS, TILE_F], F32, isOutput=False)
    wht = nc.declare_dram_parameter("wht", [B_LOC * 65536, 2], F32, isOutput=False)
    regt = nc.declare_dram_parameter("regt", [B_LOC * 65536, 2], F32, isOutput=False)
    cvals = host_constants()
    cdt = {k: (U32 if k.endswith("_u") else F32) for k in cvals}
    cparams = {k: nc.declare_dram_parameter(k, list(v.shape), cdt[k], isOutput=False)
               for k, v in cvals.items()}

    outs = [nc.declare_dram_parameter(f"out{b}", [K_OUT, 8], F32, isOutput=True)
            for b in range(B_LOC)]
    gv_out = nc.declare_dram_parameter("gv_out", [B_LOC, K_OUT], F32, isOutput=True)

    stage_v = [nc.dram_tensor(f"stage_v{b}", [128 * 8, 1], F32) for b in range(B_LOC)]
    stage_r = [nc.dram_tensor(f"stage_r{b}", [128 * 8, 1], F32) for b in range(B_LOC)]

    with TileContext(nc) as tc:
        with (
            tc.tile_pool(name="consts", bufs=1) as cpool,
            tc.tile_pool(name="xtiles", bufs=3) as xpool,
            tc.tile_pool(name="xrows", bufs=2) as xrpool,
            tc.tile_pool(name="small", bufs=2) as sp,
            tc.tile_pool(name="masks", bufs=2) as mp,
            tc.tile_pool(name="psum", bufs=2, space="PSUM") as pp,
            tc.tile_pool(name="psum_big", bufs=1, space="PSUM") as ppb,
        ):
            C = {}
            for k, v in cvals.items():
                t = cpool.tile(list(v.shape), cdt[k], tag=k)
                nc.sync.dma_start(out=t[:], in_=cparams[k][:])
                C[k] = t

            for b in range(B_LOC):
                # ---------- Phase A ----------
                cand = sp.tile([128, 8 * N_TILES], F32, tag="cand")
                for t in range(N_TILES):
                    xt = xpool.tile([128, TILE_F], F32, tag="xt")
                    nc.sync.dma_start(out=xt[:], in_=xs[b * ROWS + t * 128:
                                                       b * ROWS + (t + 1) * 128, :])
                    nc.vector.max(out=cand[:, 8 * t:8 * (t + 1)], in_=xt[:])

                # ---------- Phase B ----------
                m = sp.tile([128, 8], F32, tag="m")
                nc.vector.max(out=m[:], in_=cand[:])
                mi = sp.tile([128, 8], U32, tag="mi")
                nc.vector.max_index(out=mi[:], in_max=m[:], in_values=cand[:])

                mT_ps = pp.tile([8, 128], F32, space="PSUM", tag="ps_b")
                nc.tensor.transpose(out=mT_ps[:], in_=m[:], identity=C["c_identity"][:])
                mT = sp.tile([8, 128], F32, tag="mT")
                nc.scalar.copy(out=mT[:], in_=mT_ps[:])

                gv = sp.tile([1, K_OUT], F32, tag="gv")
                for r in range(K_OUT // 8):
                    t8 = sp.tile([8, 8], F32, tag="t8")
                    nc.vector.max(out=t8[:], in_=mT[:])
                    v64 = pp.tile([1, 64], F32, space="PSUM", tag="ps_b")
                    for j in range(8):
                        nc.tensor.matmul(v64[0:1, 8 * j:8 * (j + 1)],
                                         lhsT=C["c_sel8"][:, j:j + 1], rhs=t8[:],
                                         start=True, stop=True)
                    nc.vector.max(out=gv[0:1, 8 * r:8 * (r + 1)], in_=v64[:])
                    g8b = pp.tile([8, 8], F32, space="PSUM", tag="ps_b")
                    nc.tensor.matmul(g8b[:], lhsT=C["c_ones8"][:],
                                     rhs=gv[0:1, 8 * r:8 * (r + 1)],
                                     start=True, stop=True)
                    nc.vector.match_replace(out=mT[:], in_to_replace=g8b[:],
                                            in_values=mT[:], imm_value=NEG)
                nc.sync.dma_start(out=gv_out[b:b + 1, :], in_=gv[:])

                # ---------- Phase C ----------
                tau_b = pp.tile([128, 1], F32, space="PSUM", tag="ps_s")
                nc.tensor.transpose(out=tau_b[:],
                                    in_=gv[0:1, K_OUT - 1:K_OUT].to_broadcast([1, 128]),
                                    identity=C["c_identity"][0:1, 0:1])
                W = sp.tile([128, 8 * N_TILES], F32, tag="W")
                nc.vector.tensor_tensor(out=W[:], in0=cand[:],
                                        in1=tau_b[:].to_broadcast([128, 8 * N_TILES]),
                                        op=OP.is_ge)
                n_t = sp.tile([128, 1], F32, tag="n_t")
                nc.vector.reduce_sum(out=n_t[:], in_=W[:], axis=mybir.AxisListType.X)
                incl = pp.tile([128, 1], F32, space="PSUM", tag="ps_s")
                nc.tensor.matmul(incl[:], lhsT=C["c_triu"][:], rhs=n_t[:],
                                 start=True, stop=True)
                excl = sp.tile([128, 1], F32, tag="excl")
                nc.vector.tensor_tensor(out=excl[:], in0=incl[:], in1=n_t[:],
                                        op=OP.subtract)
                E1 = mp.tile([128, 128], F32, tag="E1")
                nc.vector.tensor_tensor(out=E1[:],
                                        in0=excl[:].to_broadcast([128, 128]),
                                        in1=C["c_irow"][:], op=OP.is_le)
                psum1 = pp.tile([128, 1], F32, space="PSUM", tag="ps_s")
                nc.tensor.matmul(psum1[:], lhsT=E1[:], rhs=C["c_ones_col"][:],
                                 start=True, stop=True)
                p_s = sp.tile([128, 1], F32, tag="p_s")
                nc.vector.tensor_scalar(out=p_s[:], in0=psum1[:], scalar1=1.0,
                                        scalar2=None, op0=OP.subtract)
                E1s = ppb.tile([128, 128], F32, space="PSUM", tag="ps_E1s")
                nc.tensor.matmul(E1s[:], lhsT=C["c_shiftup"][:], rhs=E1[:],
                                 start=True, stop=True)
                onehot = mp.tile([128, 128], F32, tag="onehot")
                nc.vector.tensor_tensor(out=onehot[:], in0=E1[:], in1=E1s[:],
                                        op=OP.subtract)
                exclps = pp.tile([128, 1], F32, space="PSUM", tag="ps_s")
                nc.tensor.matmul(exclps[:], lhsT=onehot[:], rhs=excl[:],
                                 start=True, stop=True)
                j_s = sp.tile([128, 1], F32, tag="j_s")
                nc.vector.tensor_tensor(out=j_s[:], in0=C["c_iotap_f"][:],
                                        in1=exclps[:], op=OP.subtract)
                qf = sp.tile([128, 1], F32, tag="qf")
                nc.vector.tensor_scalar(out=qf[:], in0=p_s[:], scalar1=8.0,
                                        scalar2=None, op0=OP.mult)
                nc.vector.tensor_tensor(out=qf[:], in0=qf[:], in1=j_s[:], op=OP.add)
                ctot = pp.tile([1, 1], F32, space="PSUM", tag="ps_s")
                nc.tensor.matmul(ctot[:], lhsT=n_t[:], rhs=C["c_ones_col"][:],
                                 start=True, stop=True)
                ctot_sb = sp.tile([1, 1], F32, tag="ctot_sb")
                nc.scalar.copy(out=ctot_sb[:], in_=ctot[:])
                cb = pp.tile([128, 1], F32, space="PSUM", tag="ps_s")
                nc.tensor.transpose(out=cb[:], in_=ctot_sb[:].to_broadcast([1, 128]),
                                    identity=C["c_identity"][0:1, 0:1])
                valid = sp.tile([128, 1], F32, tag="valid")
                nc.vector.tensor_tensor(out=valid[:], in0=C["c_iotap_f"][:],
                                        in1=cb[:], op=OP.is_lt)
                qv = sp.tile([128, 1], F32, tag="qv")
                nc.vector.tensor_tensor(out=qv[:], in0=qf[:], in1=valid[:], op=OP.mult)
                inv = sp.tile([128, 1], F32, tag="inv")
                nc.vector.tensor_scalar(out=inv[:], in0=valid[:], scalar1=-4096.0,
                                        scalar2=4096.0, op0=OP.mult, op1=OP.add)
                nc.vector.tensor_tensor(out=qv[:], in0=qv[:], in1=inv[:], op=OP.add)
                qu = sp.tile([128, 1], U32, tag="qu")
                nc.vector.tensor_copy(out=qu[:], in_=qv[:])

                # ---------- Phase D ----------
                t_u = sp.tile([128, 8], U32, tag="t_u")
                nc.vector.tensor_scalar(out=t_u[:], in0=mi[:], scalar1=3,
                                        scalar2=None, op0=OP.logical_shift_right)
                r_u = sp.tile([128, 8], U32, tag="r_u")
                nc.vector.tensor_scalar(out=r_u[:], in0=t_u[:], scalar1=7,
                                        scalar2=None, op0=OP.logical_shift_left)
                nc.vector.tensor_tensor(out=r_u[:], in0=r_u[:],
                                        in1=C["c_iotap_u"][:].to_broadcast([128, 8]),
                                        op=OP.add)
                rowf = sp.tile([128, 8], F32, tag="rowf")
                nc.vector.tensor_copy(out=rowf[:], in_=r_u[:])
                nc.sync.dma_start(
                    out=stage_v[b].ap().rearrange("(p j) one -> p (j one)", p=128),
                    in_=m[:])
                nc.sync.dma_start(
                    out=stage_r[b].ap().rearrange("(p j) one -> p (j one)", p=128),
                    in_=rowf[:])
                cval = sp.tile([128, 1], F32, tag="cval")
                nc.vector.memset(cval[:], NEG)
                crow = sp.tile([128, 1], F32, tag="crow")
                nc.vector.memset(crow[:], 0.0)
                nc.gpsimd.indirect_dma_start(
                    out=cval[:], out_offset=None, in_=stage_v[b].ap(),
                    in_offset=bass.IndirectOffsetOnAxis(ap=qu[:, :1], axis=0),
                    bounds_check=1023, oob_is_err=False)
                nc.gpsimd.indirect_dma_start(
                    out=crow[:], out_offset=None, in_=stage_r[b].ap(),
                    in_offset=bass.IndirectOffsetOnAxis(ap=qu[:, :1], axis=0),
                    bounds_check=1023, oob_is_err=False)

                # ---------- Phase E ----------
                rfo = sp.tile([128, 1], F32, tag="rfo")
                nc.vector.tensor_scalar(out=rfo[:], in0=crow[:], scalar1=0.0,
                                        scalar2=float(b * ROWS), op0=OP.max, op1=OP.add)
                rfo_u = sp.tile([128, 1], U32, tag="rfo_u")
                nc.vector.tensor_copy(out=rfo_u[:], in_=rfo[:])
                xrows = xrpool.tile([128, TILE_F], F32, tag="xrows")
                nc.gpsimd.indirect_dma_start(
                    out=xrows[:], out_offset=None, in_=xs[:],
                    in_offset=bass.IndirectOffsetOnAxis(ap=rfo_u[:, :1], axis=0))
                valb = sp.tile([128, 8], F32, tag="valb")
                nc.vector.tensor_copy(out=valb[:], in_=cval[:].to_broadcast([128, 8]))
                wi_u = sp.tile([128, 8], U32, tag="wi_u")
                nc.vector.max_index(out=wi_u[:], in_max=valb[:], in_values=xrows[:])
                wi_f = sp.tile([128, 8], F32, tag="wi_f")
                nc.vector.tensor_copy(out=wi_f[:], in_=wi_u[:])

                # ---------- Phase F ----------
                valT = ppb.tile([128, 128], F32, space="PSUM", tag="ps_valT")
                nc.tensor.transpose(out=valT[:], in_=cval[:].to_broadcast([128, 128]),
                                    identity=C["c_identity"][:])
                rowT = ppb.tile([128, 128], F32, space="PSUM", tag="ps_rowT")
                nc.tensor.transpose(out=rowT[:], in_=crow[:].to_broadcast([128, 128]),
                                    identity=C["c_identity"][:])
                eqv = mp.tile([128, 128], F32, tag="eqv")
                nc.vector.tensor_tensor(out=eqv[:], in0=valT[:],
                                        in1=cval[:].to_broadcast([128, 128]),
                                        op=OP.is_equal)
                eqr = mp.tile([128, 128], F32, tag="eqr")
                nc.vector.tensor_tensor(out=eqr[:], in0=rowT[:],
                                        in1=crow[:].to_broadcast([128, 128]),
                                        op=OP.is_equal)
                nc.vector.tensor_tensor(out=eqr[:], in0=eqr[:], in1=eqv[:], op=OP.mult)
                nc.vector.tensor_tensor(out=eqr[:], in0=eqr[:], in1=C["c_slt"][:],
                                        op=OP.mult)
                o_t = sp.tile([128, 1], F32, tag="o_t")
                nc.vector.reduce_sum(out=o_t[:], in_=eqr[:], axis=mybir.AxisListType.X)
                oh8 = sp.tile([128, 8], F32, tag="oh8")
                nc.vector.tensor_tensor(out=oh8[:], in0=C["c_iota08"][:],
                                        in1=o_t[:].to_broadcast([128, 8]),
                                        op=OP.is_equal)
                nc.vector.tensor_tensor(out=oh8[:], in0=oh8[:], in1=wi_f[:], op=OP.mult)
                off = sp.tile([128, 1], F32, tag="off")
                nc.vector.reduce_sum(out=off[:], in_=oh8[:], axis=mybir.AxisListType.X)
                flat = sp.tile([128, 1], F32, tag="flat")
                nc.vector.tensor_scalar(out=flat[:], in0=crow[:], scalar1=8192.0,
                                        scalar2=None, op0=OP.mult)
                nc.vector.tensor_tensor(out=flat[:], in0=flat[:], in1=off[:], op=OP.add)

                # ---------- Phase G ----------
                flatT = ppb.tile([128, 128], F32, space="PSUM", tag="ps_rowT")
                nc.tensor.transpose(out=flatT[:], in_=flat[:].to_broadcast([128, 128]),
                                    identity=C["c_identity"][:])
                gtv = mp.tile([128, 128], F32, tag="gtv")
                nc.vector.tensor_tensor(out=gtv[:], in0=valT[:],
                                        in1=cval[:].to_broadcast([128, 128]),
                                        op=OP.is_gt)
                ltf = mp.tile([128, 128], F32, tag="ltf")
                nc.vector.tensor_tensor(out=ltf[:], in0=flatT[:],
                                        in1=flat[:].to_broadcast([128, 128]),
                                        op=OP.is_lt)
                nc.vector.tensor_tensor(out=ltf[:], in0=ltf[:], in1=eqv[:], op=OP.mult)
                nc.vector.tensor_tensor(out=gtv[:], in0=gtv[:], in1=ltf[:], op=OP.add)
                rank = sp.tile([128, 1], F32, tag="rank")
                nc.vector.reduce_sum(out=rank[:], in_=gtv[:], axis=mybir.AxisListType.X)
                rank_u = sp.tile([128, 1], U32, tag="rank_u")
                nc.vector.tensor_copy(out=rank_u[:], in_=rank[:])

                # ---------- Phase H ----------
                outrow = sp.tile([128, 8], F32, tag="outrow")
                flat_u = sp.tile([128, 1], U32, tag="flat_u")
                nc.vector.tensor_copy(out=flat_u[:], in_=flat[:])
                nc.vector.tensor_scalar(out=outrow[:, 0:1], in0=flat[:],
                                        scalar1=1.0 / 65536.0, scalar2=None,
                                        op0=OP.mult)
                nc.vector.tensor_copy(out=outrow[:, 1:2], in_=cval[:])
                sidx_u = sp.tile([128, 1], U32, tag="sidx_u")
                nc.vector.tensor_scalar(out=sidx_u[:], in0=flat_u[:], scalar1=65535,
                                        scalar2=None, op0=OP.bitwise_and)
                xs_u = sp.tile([128, 1], U32, tag="xs_u")
                nc.vector.tensor_scalar(out=xs_u[:], in0=sidx_u[:], scalar1=255,
                                        scalar2=None, op0=OP.bitwise_and)
                sidx_f = sp.tile([128, 1], F32, tag="sidx_f")
                nc.vector.tensor_copy(out=sidx_f[:], in_=sidx_u[:])
                xs_f = sp.tile([128, 1], F32, tag="xs_f")
                nc.vector.tensor_copy(out=xs_f[:], in_=xs_u[:])
                ys_f = sp.tile([128, 1], F32, tag="ys_f")
                nc.vector.tensor_scalar(out=ys_f[:], in0=sidx_f[:],
                                        scalar1=1.0 / 256.0, scalar2=None,
                                        op0=OP.mult)
                soff_u = sp.tile([128, 1], U32, tag="soff_u")
                nc.vector.tensor_scalar(out=soff_u[:], in0=sidx_u[:],
                                        scalar1=b * 65536, scalar2=None, op0=OP.add)
                whg = sp.tile([128, 2], F32, tag="whg")
                nc.vector.memset(whg[:], 0.0)
                regg = sp.tile([128, 2], F32, tag="regg")
                nc.vector.memset(regg[:], 0.0)
                nc.gpsimd.indirect_dma_start(
                    out=whg[:], out_offset=None, in_=wht[:],
                    in_offset=bass.IndirectOffsetOnAxis(ap=soff_u[:, :1], axis=0))
                nc.gpsimd.indirect_dma_start(
                    out=regg[:], out_offset=None, in_=regt[:],
                    in_offset=bass.IndirectOffsetOnAxis(ap=soff_u[:, :1], axis=0))
                cx = sp.tile([128, 1], F32, tag="cx")
                nc.vector.tensor_tensor(out=cx[:], in0=xs_f[:], in1=regg[:, 0:1],
                                        op=OP.add)
                cy = sp.tile([128, 1], F32, tag="cy")
                nc.vector.tensor_tensor(out=cy[:], in0=ys_f[:], in1=regg[:, 1:2],
                                        op=OP.add)
                hw_ = sp.tile([128, 1], F32, tag="hw_")
                nc.vector.tensor_scalar(out=hw_[:], in0=whg[:, 0:1], scalar1=0.5,
                                        scalar2=None, op0=OP.mult)
                hh_ = sp.tile([128, 1], F32, tag="hh_")
                nc.vector.tensor_scalar(out=hh_[:], in0=whg[:, 1:2], scalar1=0.5,
                                        scalar2=None, op0=OP.mult)
                tmp = sp.tile([128, 1], F32, tag="tmp")
                nc.vector.tensor_tensor(out=tmp[:], in0=cx[:], in1=hw_[:],
                                        op=OP.subtract)
                nc.vector.tensor_scalar(out=outrow[:, 2:3], in0=tmp[:], scalar1=4.0,
                                        scalar2=None, op0=OP.mult)
                nc.vector.tensor_tensor(out=tmp[:], in0=cy[:], in1=hh_[:],
                                        op=OP.subtract)
                nc.vector.tensor_scalar(out=outrow[:, 3:4], in0=tmp[:], scalar1=4.0,
                                        scalar2=None, op0=OP.mult)
                nc.vector.tensor_tensor(out=tmp[:], in0=cx[:], in1=hw_[:], op=OP.add)
                nc.vector.tensor_scalar(out=outrow[:, 4:5], in0=tmp[:], scalar1=4.0,
                                        scalar2=None, op0=OP.mult)
                nc.vector.tensor_tensor(out=tmp[:], in0=cy[:], in1=hh_[:], op=OP.add)
                nc.vector.tensor_scalar(out=outrow[:, 5:6], in0=tmp[:], scalar1=4.0,
                                        scalar2=None, op0=OP.mult)
                nc.vector.tensor_copy(out=outrow[:, 6:7], in_=flat[:])
                nc.vector.tensor_copy(out=outrow[:, 7:8], in_=rank[:])

                # ---------- Phase I ----------
                nc.gpsimd.indirect_dma_start(
                    out=outs[b].ap(),
                    out_offset=bass.IndirectOffsetOnAxis(ap=rank_u[:, :1], axis=0),
                    in_=outrow[:], in_offset=None,
                    bounds_check=K_OUT - 1, oob_is_err=False)

    nc.compile()
    return nc


def shard_inputs(x, wh, reg, core):
    b0 = core * B_LOC
    m = {
        "xs": np.ascontiguousarray(
            x[b0:b0 + B_LOC].reshape(B_LOC * ROWS, TILE_F)),
        "wht": np.ascontiguousarray(
            wh[b0:b0 + B_LOC].transpose(0, 2, 3, 1).reshape(B_LOC * 65536, 2)),
        "regt": np.ascontiguousarray(
            reg[b0:b0 + B_LOC].transpose(0, 2, 3, 1).reshape(B_LOC * 65536, 2)),
    }
    m.update(host_constants())
    return m


def _register_ntff_hook():
    """Make trace=True work under axon when antenv.axon_hooks is missing."""
    try:
        from antenv.axon_hooks import get_axon_ntff_profile_hook  # noqa: F401
        return
    except ImportError:
        pass
    try:
        import sys
        import types
        import trn_agent_boot.trn_boot as tb
        mod = types.ModuleType("antenv.axon_hooks")
        hook = [tb._ntff_profile_via_ctypes('/opt/axon/libaxon_pjrt.so')]
        mod.set_axon_ntff_profile_hook = lambda h: hook.__setitem__(0, h)
        mod.get_axon_ntff_profile_hook = lambda: hook[0]
        sys.modules["antenv.axon_hooks"] = mod
        import antenv
        antenv.axon_hooks = mod
    except Exception:
        pass


_NC_CACHE = {}


def kernel(x, wh, reg):
    global LAST_EXEC_NS
    x = np.ascontiguousarray(x, dtype=np.float32)
    wh = np.ascontiguousarray(wh, dtype=np.float32)
    reg = np.ascontiguousarray(reg, dtype=np.float32)
    assert x.shape == (B_FULL, 80, 256, 256)

    if "nc" not in _NC_CACHE:
        _NC_CACHE["nc"] = build_kernel(N_CORES)
    nc = _NC_CACHE["nc"]

    in_maps = [shard_inputs(x, wh, reg, c) for c in range(N_CORES)]
    trace = os.environ.get("CK_TRACE", "0") == "1"
    if trace:
        _register_ntff_hook()
    res = run_bass_kernel_spmd(nc, in_maps, list(range(N_CORES)), trace=trace)
    LAST_EXEC_NS = res.exec_time_ns

    classes = np.zeros((B_FULL, TOPK), np.float32)
    scores = np.zeros((B_FULL, TOPK), np.float32)
    boxes = np.zeros((B_FULL, TOPK, 4), np.float32)
    for c in range(N_CORES):
        for b in range(B_LOC):
            out = res.results[c][f"out{b}"]
            g = c * B_LOC + b
            classes[g] = out[:TOPK, 0]
            scores[g] = out[:TOPK, 1]
            boxes[g] = out[:TOPK, 2:6]
    return classes, scores, boxes
